# revision 5
# baseline (speedup 1.0000x reference)
"""Trainium2 Bass kernel for BertSelfAttention (B=1, S=4096, HID=768, 12 heads).

Sharding: 8 cores = 4 head-groups x 2 query-halves. Each core computes 3 heads
for 2048 query rows against all 4096 keys, fused (scores never hit HBM).

v2 design (vs v1 baseline):
  - Score matmuls are row-tiled 64x128 (tile_position): head-lo lives on SBUF
    partitions 0:64, head-hi on 64:128, and the two 64-contraction score
    matmuls run CONCURRENTLY on the two PE row-halves (2x score throughput,
    no zero-padding waste, no memsets).
  - Heads 0/1 pair naturally (h0 lower / h1 upper, the v1 layout). Head 2 is
    duplicated onto both halves via duplicated weight columns (reusing the
    v1 zero-pad columns, so projection cost is unchanged) and pairs with
    itself across two query blocks.
  - One [128,1024] fp32 PSUM score tile per key-chunk holds BOTH paired
    heads' scores (T0 writes cols 0:512 = bank A, T8 writes 512:1024 = bank
    B), so one ScalarE exp covers both heads.
  - hsqT input dropped: per-core hsT is key-permuted so the core's own query
    rows are always columns 0:2048 (softmax is permutation-invariant over
    keys); q-projections just index hsT.
  - PSUM: sc 2x2 banks + cx_lo + cx_hi + 2 proj banks = 8 exactly.

Per-core dataflow otherwise follows v1: bf16 matmuls, fp32 PSUM, additive
mask handled by scaling V rows (and the appended ones-column) with exp(mask),
V augmented with a ones column per head so the context matmul accumulates the
softmax denominator for free, ctx^T tiles PE-transposed back to [q, d] and
divided by the denominator on VectorE.
"""

import sys

sys.path.insert(0, "/opt/trn_rl_repo")

import ml_dtypes
import numpy as np

import concourse.bacc as bacc
import concourse.mybir as mybir
import concourse.tile as tile
from concourse import bass_utils

B, S, HID = 1, 4096, 768
NH, HD = 12, 64
N_CORES = 8
HG = 4  # head-groups (tensor parallel)
QS = 2  # query splits (data parallel on sequence)
HPC = NH // HG  # 3 heads per core
SQ = S // QS  # 2048 query rows per core
CC = HPC * HD  # 192 projection columns per core
WCC = 256  # weight cols per chunk in wqb/wkb: [h0|h1|h2|h2]
VC = HPC * (HD + 1)  # 195 augmented V columns (ones col per head)
NHC = HID // 128  # 6 contraction chunks
NT = S // 128  # 32 key tiles
NJ = SQ // 512  # 4 query blocks

f32 = mybir.dt.float32
bf16 = mybir.dt.bfloat16
bf16np = ml_dtypes.bfloat16

# pair-blocks: (h_lo, h_hi, j_lo, j_hi, pair_idx) where pair_idx 0 = heads
# 0/1 (kt01/qt01), 1 = head 2 duplicated (kt2/qt2)
PBS = [(0, 1, j, j, 0) for j in range(NJ)] + [(2, 2, 0, 1, 1), (2, 2, 2, 3, 1)]

_CACHE = {}


def _build():
    EXP = mybir.ActivationFunctionType.Exp
    nc = bacc.Bacc("TRN2", target_bir_lowering=False)

    hsT_d = nc.dram_tensor("hsT", [HID, S], bf16, kind="ExternalInput")
    wqb_d = nc.dram_tensor("wqb", [128, NHC * WCC], bf16, kind="ExternalInput")
    wkb_d = nc.dram_tensor("wkb", [128, NHC * WCC], bf16, kind="ExternalInput")
    wvb_d = nc.dram_tensor("wvb", [128, NHC * VC], bf16, kind="ExternalInput")
    bqt_d = nc.dram_tensor("bqt", [128, HPC], f32, kind="ExternalInput")
    bkt_d = nc.dram_tensor("bkt", [128, HPC], f32, kind="ExternalInput")
    bvb_d = nc.dram_tensor("bvb", [1, VC], bf16, kind="ExternalInput")
    maskt_d = nc.dram_tensor("maskt", [128, NT], f32, kind="ExternalInput")
    ident_d = nc.dram_tensor("ident", [128, 128], f32, kind="ExternalInput")
    out_d = nc.dram_tensor("out", [SQ, CC], f32, kind="ExternalOutput")

    with tile.TileContext(nc) as tc:
        with (
            tc.tile_pool(name="persist", bufs=1) as P,
            tc.tile_pool(name="work", bufs=4) as WK,
            tc.tile_pool(name="outp", bufs=2) as OP,
            tc.tile_pool(name="scp", bufs=2, space="PSUM") as SCP,
            tc.tile_pool(name="cxp", bufs=1, space="PSUM") as CP,
            tc.tile_pool(name="ppsum", bufs=2, space="PSUM") as PP,
        ):
            # ---- persistent SBUF tensors ----
            # chunk-major transposed activations: chunk c at cols [c*S, (c+1)*S)
            hsT = P.tile([128, NHC * S], bf16, tag="hsT")
            wqb = P.tile([128, NHC * WCC], bf16, tag="wqb")
            wkb = P.tile([128, NHC * WCC], bf16, tag="wkb")
            wvb = P.tile([128, NHC * VC], bf16, tag="wvb")
            bvb = P.tile([1, VC], bf16, tag="bvb")
            bqt = P.tile([128, HPC], f32, tag="bqt")
            bkt = P.tile([128, HPC], f32, tag="bkt")
            maskt = P.tile([128, NT], f32, tag="maskt")
            wmask = P.tile([128, NT], f32, tag="wmask")
            identf = P.tile([128, 128], f32, tag="identf")
            onesb = P.tile([1, 128], bf16, tag="onesb")
            # kt/qt: pair 0 = h0 on partitions 0:64 + h1 on 64:128;
            # pair 1 = h2 duplicated on both halves
            kts = [
                P.tile([128, S], bf16, tag=f"kt{p}", name=f"kt{p}")
                for p in range(2)
            ]
            qts = [
                P.tile([128, SQ], bf16, tag=f"qt{p}", name=f"qt{p}")
                for p in range(2)
            ]
            vv = P.tile([128, NT * VC], bf16, tag="vv")

            # ---- DMA helpers ----
            hsT_3d = hsT.rearrange("p (c s) -> p c s", s=S)
            hsT_d3 = hsT_d.rearrange("(c p) s -> p c s", p=128)

            def load_hsT_cols(s0, s1):
                nc.sync.dma_start(hsT_3d[:, :, s0:s1], hsT_d3[:, :, s0:s1])

            # ---- q/k projection units ----
            # one paired matmul chain produces both partition halves:
            # pair 0 -> stationary cols 0:128 of each chunk ([h0|h1]),
            # pair 1 -> cols 128:256 ([h2|h2])
            def emit_qk_mm(kind, pi, j, c, ps):
                wsrc = wqb if kind == "qt" else wkb
                coff = 128 * pi
                nc.tensor.matmul(
                    ps[:],
                    wsrc[:, c * WCC + coff : c * WCC + coff + 128],
                    hsT[:, c * S + j * 512 : c * S + (j + 1) * 512],
                    start=(c == 0),
                    stop=(c == NHC - 1),
                )

            def emit_qk_finish(kind, pi, j, ps):
                dst = (qts if kind == "qt" else kts)[pi]
                bias = bqt if kind == "qt" else bkt
                b0, b1 = (0, 1) if pi == 0 else (2, 2)
                nc.vector.tensor_scalar_add(
                    dst[0:64, j * 512 : (j + 1) * 512],
                    ps[0:64, :],
                    bias[0:64, b0 : b0 + 1],
                )
                nc.vector.tensor_scalar_add(
                    dst[64:128, j * 512 : (j + 1) * 512],
                    ps[64:128, :],
                    bias[64:128, b1 : b1 + 1],
                )

            def qk_unit(kind, pi, j):
                ps = PP.tile([128, 512], f32, tag="proj", name="ps")
                for c in range(NHC):
                    emit_qk_mm(kind, pi, j, c, ps)
                emit_qk_finish(kind, pi, j, ps)

            # stepwise projection queue: one matmul per call so bursts never
            # overrun the per-tile PE slack
            proj_q = []

            def enqueue_proj(kind, pi, j):
                proj_q.append({"kind": kind, "pi": pi, "j": j, "step": 0})

            def proj_step():
                if not proj_q:
                    return
                st = proj_q[0]
                c = st["step"]
                if c == 0:
                    st["ps"] = PP.tile([128, 512], f32, tag="proj", name="ps")
                emit_qk_mm(st["kind"], st["pi"], st["j"], c, st["ps"])
                if c == NHC - 1:
                    emit_qk_finish(st["kind"], st["pi"], st["j"], st["ps"])
                    proj_q.pop(0)
                else:
                    st["step"] += 1

            def v_unit(t):
                pv = PP.tile([128, VC], f32, tag="proj", name="pv")
                for c in range(NHC):
                    nc.tensor.matmul(
                        pv[:],
                        hsT[:, c * S + t * 128 : c * S + (t + 1) * 128],
                        wvb[:, c * VC : (c + 1) * VC],
                        start=(c == 0),
                        stop=False,
                    )
                nc.tensor.matmul(pv[:], onesb[:], bvb[:], start=False, stop=True)
                nc.vector.tensor_scalar_mul(
                    vv[:, t * VC : (t + 1) * VC], pv[:], wmask[:, t : t + 1]
                )

            # ---- deferred out-stage, pipelined into the next block ----
            out_stage_q = []

            def emit_out_stage():
                if not out_stage_q:
                    return
                # prioritize step-0 (the DVE copy that frees the cx PSUM
                # bank) of every queued entry, so the next block's ctx
                # accumulation never waits long on the bank
                entry = None
                for e in out_stage_q:
                    if e[3]["step"] == 0:
                        entry = e
                        break
                if entry is None:
                    entry = out_stage_q[0]
                _advance_out_stage(entry)

            def _advance_out_stage(entry):
                jq, h, cx, st = entry
                if st["step"] == 0:
                    cs = OP.tile([65, 512], f32, tag="cs", name="cs")
                    nc.vector.tensor_copy(cs[:], cx[:])
                    st["cs"] = cs
                    st["ot"] = OP.tile([128, 4 * 64], f32, tag="ot", name="ot")
                elif st["step"] == 1:
                    cs = st["cs"]
                    tp2 = PP.tile([128, 4 * 65], f32, tag="proj", name="tp2")
                    st["tp2"] = tp2
                    for t4 in range(4):
                        nc.tensor.transpose(
                            tp2[:, t4 * 65 : (t4 + 1) * 65],
                            cs[:, t4 * 128 : (t4 + 1) * 128],
                            identf[0:65, 0:65],
                        )
                elif st["step"] <= 5:
                    t4 = st["step"] - 2
                    tp2, ot = st["tp2"], st["ot"]
                    rc = OP.tile([128, 1], f32, tag="rc", name="rc")
                    nc.vector.reciprocal(rc[:], tp2[:, t4 * 65 + 64 : t4 * 65 + 65])
                    nc.vector.tensor_scalar_mul(
                        ot[:, t4 * 64 : (t4 + 1) * 64],
                        tp2[:, t4 * 65 : t4 * 65 + 64],
                        rc[:],
                    )
                    if t4 == 3:
                        dst = out_d[
                            jq * 512 : (jq + 1) * 512, h * 64 : (h + 1) * 64
                        ].rearrange("(t p) d -> p t d", p=128)
                        nc.sync.dma_start(
                            dst, ot.rearrange("p (t d) -> p t d", d=64)
                        )
                        out_stage_q.pop(0)
                        return
                st["step"] += 1

            def flush_out_stages():
                while out_stage_q:
                    emit_out_stage()

            # ---- ramp: pipelined input loads + first-needed projections ----
            # mask load + exp first: ScalarE is in-order, so this tiny
            # ACTIVATE must clear the queue before the first score exp
            nc.sync.dma_start(maskt[:], maskt_d[:])
            nc.scalar.activation(wmask[:], maskt[:], EXP)
            nc.vector.memset(onesb[:], 1.0)
            nc.sync.dma_start(wqb[:], wqb_d[:])
            nc.sync.dma_start(bqt[:], bqt_d[:])
            load_hsT_cols(0, 512)
            nc.sync.dma_start(wkb[:], wkb_d[:])
            nc.sync.dma_start(bkt[:], bkt_d[:])
            qk_unit("qt", 0, 0)
            qk_unit("kt", 0, 0)
            load_hsT_cols(512, 1024)
            nc.sync.dma_start(wvb[:], wvb_d[:])
            nc.sync.dma_start(bvb[:], bvb_d[:])
            nc.sync.dma_start(identf[:], ident_d[:])
            load_hsT_cols(1024, 2048)
            load_hsT_cols(2048, 4096)

            # per-pair-block projection enqueue schedule (ready just in time;
            # pair-block 0 instead emits immediate whole-unit bursts)
            pb_enqueue = {
                1: [("qt", 0, 2)] + [("kt", 1, j) for j in range(4)],
                2: [("qt", 0, 3)] + [("kt", 1, j) for j in range(4, 8)],
                3: [("qt", 1, 0), ("qt", 1, 1)],
                4: [("qt", 1, 2), ("qt", 1, 3)],
            }

            pending_final = None

            for pb_idx, (h_lo, h_hi, j_lo, j_hi, pi) in enumerate(PBS):
                ktA, qtA = kts[pi], qts[pi]
                for item in pb_enqueue.get(pb_idx, []):
                    enqueue_proj(*item)
                cx_lo = CP.tile([65, 512], f32, tag="cxlo", name="cxlo")
                cx_hi = CP.tile([65, 512], f32, tag="cxhi", name="cxhi")
                pts = []

                def emit_ctx(g, pts=pts, cx_lo=cx_lo, cx_hi=cx_hi,
                             h_lo=h_lo, h_hi=h_hi):
                    pt = pts[g]
                    nc.tensor.matmul(
                        cx_lo[:],
                        vv[:, g * VC + h_lo * 65 : g * VC + h_lo * 65 + 65],
                        pt[:, 0:512],
                        start=(g == 0),
                        stop=(g == NT - 1),
                    )
                    nc.tensor.matmul(
                        cx_hi[:],
                        vv[:, g * VC + h_hi * 65 : g * VC + h_hi * 65 + 65],
                        pt[:, 512:1024],
                        start=(g == 0),
                        stop=(g == NT - 1),
                    )

                for t in range(NT):
                    # scores for key chunk t, both heads, row-tiled 64x128:
                    # T0 (partitions 0:64) -> cols 0:512 (bank A),
                    # T8 (partitions 64:128) -> cols 512:1024 (bank B)
                    sc = SCP.tile([128, 1024], f32, tag="sc", name="sc")
                    nc.tensor.matmul(
                        sc[:, 0:512],
                        ktA[0:64, t * 128 : (t + 1) * 128],
                        qtA[0:64, j_lo * 512 : (j_lo + 1) * 512],
                        start=True,
                        stop=True,
                    )
                    nc.tensor.matmul(
                        sc[:, 512:1024],
                        ktA[64:128, t * 128 : (t + 1) * 128],
                        qtA[64:128, j_hi * 512 : (j_hi + 1) * 512],
                        start=True,
                        stop=True,
                    )
                    pt = WK.tile([128, 1024], bf16, tag="pt", name="pt")
                    nc.scalar.activation(pt[:], sc[:], EXP, scale=0.125)
                    pts.append(pt)
                    if t == 0 and pending_final is not None:
                        pending_final()
                        pending_final = None
                    emit_out_stage()
                    if t == 0:
                        emit_out_stage()  # free both cx banks right away
                    # interleave projections/V into the activation-bound
                    # steady state (after the exp emission so scores are
                    # never delayed behind projection work)
                    if pb_idx == 0:
                        v_unit(t)
                        if t % 2 == 0 and t // 2 + 1 <= 7:
                            qk_unit("kt", 0, t // 2 + 1)
                        if t == 24:
                            qk_unit("qt", 0, 1)
                    else:
                        proj_step()
                    # ctx runs one chunk behind exp so the PE overlaps the
                    # activation latency with the previous chunk's ctx
                    if t > 0:
                        emit_ctx(t - 1)
                # final chunk's ctx is deferred into the next block so the
                # transition never stalls on the last exp
                pending_final = (lambda f=emit_ctx: f(NT - 1))
                out_stage_q.append((j_lo, h_lo, cx_lo, {"step": 0}))
                out_stage_q.append((j_hi, h_hi, cx_hi, {"step": 0}))
            if pending_final is not None:
                pending_final()
                pending_final = None
            flush_out_stages()

    nc.compile()
    return nc


def _get_nc():
    if "nc" not in _CACHE:
        _CACHE["nc"] = _build()
    return _CACHE["nc"]


def _in_maps(hs, mask, Wq, bq, Wk, bk, Wv, bv):
    ident = np.eye(128, dtype=np.float32)

    def qk_chunks(W, hg):  # [768, :] f32 -> [128, 6*256] bf16: [h0|h1|h2|h2]
        out = np.empty((128, NHC * WCC), bf16np)
        for c in range(NHC):
            blk = W[c * 128 : (c + 1) * 128, hg * CC : (hg + 1) * CC].astype(
                bf16np
            )
            out[:, c * WCC : c * WCC + CC] = blk
            out[:, c * WCC + CC : c * WCC + WCC] = blk[:, 2 * HD : 3 * HD]
        return out

    def v_chunks(W):  # augmented V weights -> [128, 6*195] bf16
        out = np.empty((128, NHC * VC), bf16np)
        for c in range(NHC):
            out[:, c * VC : (c + 1) * VC] = W[c * 128 : (c + 1) * 128, :].astype(
                bf16np
            )
        return out

    # per query-half: key order permuted so own queries are keys 0:2048
    m32 = mask.reshape(NT, 128)
    hsT_sh = []
    maskt_sh = []
    for sh in range(QS):
        perm = np.roll(np.arange(S), -sh * SQ)
        hsT_sh.append(np.ascontiguousarray(hs[perm, :].astype(bf16np).T))
        maskt_sh.append(
            np.ascontiguousarray(np.roll(m32, -sh * (NT // QS), axis=0).T)
        )

    maps = []
    for core in range(N_CORES):
        hg, sh = core // QS, core % QS
        wv_aug = np.zeros((HID, VC), np.float32)
        bv_aug = np.zeros((1, VC), np.float32)
        for h in range(HPC):
            wv_aug[:, h * 65 : h * 65 + 64] = Wv[
                :, hg * CC + h * 64 : hg * CC + (h + 1) * 64
            ]
            bv_aug[0, h * 65 : h * 65 + 64] = bv[
                hg * CC + h * 64 : hg * CC + (h + 1) * 64
            ]
            bv_aug[0, h * 65 + 64] = 1.0
        # per-head bias columns: col 0 = h0 (lower half), col 1 = h1 (upper
        # half), col 2 = h2 (both halves, duplicated layout)
        bqt = np.zeros((128, HPC), np.float32)
        bkt = np.zeros((128, HPC), np.float32)
        for h, lo in ((0, 0), (1, 64)):
            bqt[lo : lo + 64, h] = bq[hg * CC + h * 64 : hg * CC + (h + 1) * 64]
            bkt[lo : lo + 64, h] = bk[hg * CC + h * 64 : hg * CC + (h + 1) * 64]
        for lo in (0, 64):
            bqt[lo : lo + 64, 2] = bq[hg * CC + 2 * 64 : hg * CC + 3 * 64]
            bkt[lo : lo + 64, 2] = bk[hg * CC + 2 * 64 : hg * CC + 3 * 64]
        maps.append(
            {
                "hsT": hsT_sh[sh],
                "wqb": qk_chunks(Wq, hg),
                "wkb": qk_chunks(Wk, hg),
                "wvb": v_chunks(wv_aug),
                "bqt": bqt,
                "bkt": bkt,
                "bvb": bv_aug.astype(bf16np),
                "maskt": maskt_sh[sh],
                "ident": ident,
            }
        )
    return maps


def kernel(hidden_states, attention_mask, Wq, bq, Wk, bk, Wv, bv, **run_kwargs):
    hs = np.ascontiguousarray(np.asarray(hidden_states, np.float32).reshape(S, HID))
    mask = np.ascontiguousarray(np.asarray(attention_mask, np.float32).reshape(S))
    Wq = np.asarray(Wq, np.float32)
    Wk = np.asarray(Wk, np.float32)
    Wv = np.asarray(Wv, np.float32)
    bq = np.asarray(bq, np.float32)
    bk = np.asarray(bk, np.float32)
    bv = np.asarray(bv, np.float32)

    nc = _get_nc()
    maps = _in_maps(hs, mask, Wq, bq, Wk, bk, Wv, bv)
    res = bass_utils.run_bass_kernel_spmd(
        nc, maps, core_ids=list(range(N_CORES)), **run_kwargs
    )
    out = np.zeros((S, NH * HD), np.float32)
    for core in range(N_CORES):
        hg, sh = core // QS, core % QS
        out[sh * SQ : (sh + 1) * SQ, hg * CC : (hg + 1) * CC] = res.results[core][
            "out"
        ]
    if "trace" in run_kwargs:
        _CACHE["last_result"] = res
    return out.reshape(B, S, NH * HD)


# revision 13
# speedup vs baseline: 1.1767x; 1.1767x over previous
"""Trainium2 Bass kernel for BertSelfAttention (B=1, S=4096, HID=768, 12 heads).

Sharding: 8 cores = 4 head-groups x 2 query-halves. Each core computes 3 heads
for 2048 query rows against all 4096 keys, fused (scores never hit HBM).

v3 design (vs v1 baseline):
  - Two head-blocks are processed as a PAIR, key chunk by key chunk: one
    [128,1024] fp32 PSUM score tile holds BOTH blocks' scores for a chunk
    (lo block cols 0:512, hi block 512:1024) so ONE ScalarE exp per chunk
    covers both blocks. ScalarE (the bottleneck engine, ~1.15us per exp)
    runs back-to-back with zero idle in steady state.
  - Heads 0/1 pair per query block; head 2 pairs with itself across two
    query blocks. Score matmuls stay plain full-128 contractions with
    zero-padded halves: a row-tiled 64x128 variant was measured FASTER on
    the PE but pushed the chip into the P0 power state (all clocks x5/6),
    slowing the bottleneck ScalarE - zero halves toggle nothing and are
    power-cheap.
  - hsqT input dropped: per-core hsT is key-permuted so the core's own query
    rows are always columns 0:2048 (softmax is permutation-invariant over
    keys); q-projections just index hsT.
  - PSUM: sc 2x2 banks + cx_lo + cx_hi + 2 proj banks = 8 exactly.
  - kt/qt zero halves are written just-in-time by small DVE memsets per
    projection unit (no big ramp memsets).

Per-core dataflow otherwise follows v1: bf16 matmuls, fp32 PSUM, additive
mask handled by scaling V rows (and the appended ones-column) with exp(mask),
V augmented with a ones column per head so the context matmul accumulates the
softmax denominator for free, ctx^T tiles PE-transposed back to [q, d] and
divided by the denominator on VectorE.
"""

import sys

sys.path.insert(0, "/opt/trn_rl_repo")

import ml_dtypes
import numpy as np

import concourse.bacc as bacc
import concourse.mybir as mybir
import concourse.tile as tile
from concourse import bass_utils

B, S, HID = 1, 4096, 768
NH, HD = 12, 64
N_CORES = 8
HG = 4  # head-groups (tensor parallel)
QS = 2  # query splits (data parallel on sequence)
HPC = NH // HG  # 3 heads per core
SQ = S // QS  # 2048 query rows per core
CC = HPC * HD  # 192 projection columns per core
WCC = 256  # weight cols per chunk in wqb/wkb: [h0|h1|h2|h2]
VC = HPC * (HD + 1)  # 195 augmented V columns (ones col per head)
NHC = HID // 128  # 6 contraction chunks
NT = S // 128  # 32 key tiles
NJ = SQ // 512  # 4 query blocks

f32 = mybir.dt.float32
bf16 = mybir.dt.bfloat16
bf16np = ml_dtypes.bfloat16

# pair-blocks: (h_lo, h_hi, j_lo, j_hi) — heads 0/1 pair per query block,
# head 2 pairs with itself across two query blocks
PBS = [(0, 1, j, j) for j in range(NJ)] + [(2, 2, 0, 1), (2, 2, 2, 3)]

_CACHE = {}


def _build():
    EXP = mybir.ActivationFunctionType.Exp
    nc = bacc.Bacc("TRN2", target_bir_lowering=False)

    hsT_d = nc.dram_tensor("hsT", [HID, S], bf16, kind="ExternalInput")
    wqb_d = nc.dram_tensor("wqb", [128, NHC * WCC], bf16, kind="ExternalInput")
    wkb_d = nc.dram_tensor("wkb", [128, NHC * WCC], bf16, kind="ExternalInput")
    wvb_d = nc.dram_tensor("wvb", [128, NHC * VC], bf16, kind="ExternalInput")
    bqt_d = nc.dram_tensor("bqt", [128, HPC], f32, kind="ExternalInput")
    bkt_d = nc.dram_tensor("bkt", [128, HPC], f32, kind="ExternalInput")
    bvb_d = nc.dram_tensor("bvb", [1, VC], bf16, kind="ExternalInput")
    maskt_d = nc.dram_tensor("maskt", [128, NT], f32, kind="ExternalInput")
    ident_d = nc.dram_tensor("ident", [128, 128], f32, kind="ExternalInput")
    out_d = nc.dram_tensor("out", [SQ, CC], f32, kind="ExternalOutput")

    with tile.TileContext(nc) as tc:
        with (
            tc.tile_pool(name="persist", bufs=1) as P,
            tc.tile_pool(name="work", bufs=4) as WK,
            tc.tile_pool(name="outp", bufs=2) as OP,
            tc.tile_pool(name="scp", bufs=2, space="PSUM") as SCP,
            tc.tile_pool(name="cxp", bufs=1, space="PSUM") as CP,
            tc.tile_pool(name="ppsum", bufs=2, space="PSUM") as PP,
        ):
            # ---- persistent SBUF tensors ----
            # chunk-major transposed activations: chunk c at cols [c*S, (c+1)*S)
            hsT = P.tile([128, NHC * S], bf16, tag="hsT")
            wqb = P.tile([128, NHC * WCC], bf16, tag="wqb")
            wkb = P.tile([128, NHC * WCC], bf16, tag="wkb")
            wvb = P.tile([128, NHC * VC], bf16, tag="wvb")
            bvb = P.tile([1, VC], bf16, tag="bvb")
            bqt = P.tile([128, HPC], f32, tag="bqt")
            bkt = P.tile([128, HPC], f32, tag="bkt")
            maskt = P.tile([128, NT], f32, tag="maskt")
            wmask = P.tile([128, NT], f32, tag="wmask")
            identf = P.tile([128, 128], f32, tag="identf")
            onesb = P.tile([1, 128], bf16, tag="onesb")
            # per-head K^T/Q^T: head h occupies partitions H_LO[h]:H_LO[h]+64,
            # the other half is zero (written by the zero-padded projection,
            # no memsets) so full-128 score contractions are exact and
            # power-cheap (the zero half toggles nothing in the PE array)
            kts = [
                P.tile([128, S], bf16, tag=f"kt{h}", name=f"kt{h}")
                for h in range(HPC)
            ]
            qts = [
                P.tile([128, SQ], bf16, tag=f"qt{h}", name=f"qt{h}")
                for h in range(HPC)
            ]
            vv = P.tile([128, NT * VC], bf16, tag="vv")

            # ---- DMA helpers ----
            hsT_3d = hsT.rearrange("p (c s) -> p c s", s=S)
            hsT_d3 = hsT_d.rearrange("(c p) s -> p c s", p=128)

            def load_hsT_cols(s0, s1):
                nc.sync.dma_start(hsT_3d[:, :, s0:s1], hsT_d3[:, :, s0:s1])

            # ---- q/k projection units ----
            # one paired matmul chain produces both partition halves:
            # pair 0 -> stationary cols 0:128 of each chunk ([h0|h1]),
            # pair 1 -> cols 128:256 ([h2|h2])
            def emit_qk_mm(kind, pi, j, c, ps):
                wsrc = wqb if kind == "qt" else wkb
                coff = 128 * pi
                nc.tensor.matmul(
                    ps[:],
                    wsrc[:, c * WCC + coff : c * WCC + coff + 128],
                    hsT[:, c * S + j * 512 : c * S + (j + 1) * 512],
                    start=(c == 0),
                    stop=(c == NHC - 1),
                )

            def emit_qk_finish(kind, pi, j, ps):
                dsts = qts if kind == "qt" else kts
                bias = bqt if kind == "qt" else bkt
                blk = slice(j * 512, (j + 1) * 512)
                if pi == 0:
                    nc.vector.tensor_scalar_add(
                        dsts[0][0:64, blk], ps[0:64, :], bias[0:64, 0:1]
                    )
                    nc.vector.tensor_scalar_add(
                        dsts[1][64:128, blk], ps[64:128, :], bias[64:128, 1:2]
                    )
                    # zero the complementary halves just-in-time (power-cheap
                    # full-128 contraction needs them zero)
                    nc.vector.memset(dsts[0][64:128, blk], 0.0)
                    nc.vector.memset(dsts[1][0:64, blk], 0.0)
                else:
                    # h2: upper weight cols are zero-padded, so the upper add
                    # writes zeros (+ zero bias) — both halves covered
                    nc.vector.tensor_scalar_add(
                        dsts[2][0:64, blk], ps[0:64, :], bias[0:64, 2:3]
                    )
                    nc.vector.tensor_scalar_add(
                        dsts[2][64:128, blk], ps[64:128, :], bias[64:128, 2:3]
                    )

            def qk_unit(kind, pi, j):
                ps = PP.tile([128, 512], f32, tag="proj", name="ps")
                for c in range(NHC):
                    emit_qk_mm(kind, pi, j, c, ps)
                emit_qk_finish(kind, pi, j, ps)

            # stepwise projection queue: one matmul per call so bursts never
            # overrun the per-tile PE slack
            proj_q = []

            def enqueue_proj(kind, pi, j):
                proj_q.append({"kind": kind, "pi": pi, "j": j, "step": 0})

            def proj_step():
                if not proj_q:
                    return
                st = proj_q[0]
                c = st["step"]
                if c == 0:
                    st["ps"] = PP.tile([128, 512], f32, tag="proj", name="ps")
                emit_qk_mm(st["kind"], st["pi"], st["j"], c, st["ps"])
                if c == NHC - 1:
                    emit_qk_finish(st["kind"], st["pi"], st["j"], st["ps"])
                    proj_q.pop(0)
                else:
                    st["step"] += 1

            def v_unit(t):
                pv = PP.tile([128, VC], f32, tag="proj", name="pv")
                for c in range(NHC):
                    nc.tensor.matmul(
                        pv[:],
                        hsT[:, c * S + t * 128 : c * S + (t + 1) * 128],
                        wvb[:, c * VC : (c + 1) * VC],
                        start=(c == 0),
                        stop=False,
                    )
                nc.tensor.matmul(pv[:], onesb[:], bvb[:], start=False, stop=True)
                nc.vector.tensor_scalar_mul(
                    vv[:, t * VC : (t + 1) * VC], pv[:], wmask[:, t : t + 1]
                )

            # ---- deferred out-stage, pipelined into the next block ----
            out_stage_q = []

            def emit_out_stage():
                if not out_stage_q:
                    return
                # prioritize step-0 (the DVE copy that frees the cx PSUM
                # bank) of every queued entry, so the next block's ctx
                # accumulation never waits long on the bank
                entry = None
                for e in out_stage_q:
                    if e[3]["step"] == 0:
                        entry = e
                        break
                if entry is None:
                    entry = out_stage_q[0]
                _advance_out_stage(entry)

            def _advance_out_stage(entry):
                jq, h, cx, st = entry
                if st["step"] == 0:
                    cs = OP.tile([65, 512], f32, tag="cs", name="cs")
                    nc.vector.tensor_copy(cs[:], cx[:])
                    st["cs"] = cs
                    st["ot"] = OP.tile([128, 4 * 64], f32, tag="ot", name="ot")
                elif st["step"] == 1:
                    cs = st["cs"]
                    tp2 = PP.tile([128, 4 * 65], f32, tag="proj", name="tp2")
                    st["tp2"] = tp2
                    for t4 in range(4):
                        nc.tensor.transpose(
                            tp2[:, t4 * 65 : (t4 + 1) * 65],
                            cs[:, t4 * 128 : (t4 + 1) * 128],
                            identf[0:65, 0:65],
                        )
                elif st["step"] <= 5:
                    t4 = st["step"] - 2
                    tp2, ot = st["tp2"], st["ot"]
                    rc = OP.tile([128, 1], f32, tag="rc", name="rc")
                    nc.vector.reciprocal(rc[:], tp2[:, t4 * 65 + 64 : t4 * 65 + 65])
                    nc.vector.tensor_scalar_mul(
                        ot[:, t4 * 64 : (t4 + 1) * 64],
                        tp2[:, t4 * 65 : t4 * 65 + 64],
                        rc[:],
                    )
                    if t4 == 3:
                        dst = out_d[
                            jq * 512 : (jq + 1) * 512, h * 64 : (h + 1) * 64
                        ].rearrange("(t p) d -> p t d", p=128)
                        nc.sync.dma_start(
                            dst, ot.rearrange("p (t d) -> p t d", d=64)
                        )
                        out_stage_q.pop(0)
                        return
                st["step"] += 1

            def flush_out_stages():
                while out_stage_q:
                    emit_out_stage()

            # ---- ramp: pipelined input loads + first-needed projections ----
            # mask load + exp first: ScalarE is in-order, so this tiny
            # ACTIVATE must clear the queue before the first score exp
            nc.sync.dma_start(maskt[:], maskt_d[:])
            nc.scalar.activation(wmask[:], maskt[:], EXP)
            nc.vector.memset(onesb[:], 1.0)
            nc.sync.dma_start(wqb[:], wqb_d[:])
            nc.sync.dma_start(bqt[:], bqt_d[:])
            load_hsT_cols(0, 512)
            nc.sync.dma_start(wkb[:], wkb_d[:])
            nc.sync.dma_start(bkt[:], bkt_d[:])
            qk_unit("qt", 0, 0)
            qk_unit("kt", 0, 0)
            load_hsT_cols(512, 1024)
            nc.sync.dma_start(wvb[:], wvb_d[:])
            nc.sync.dma_start(bvb[:], bvb_d[:])
            nc.sync.dma_start(identf[:], ident_d[:])
            load_hsT_cols(1024, 2048)
            load_hsT_cols(2048, 4096)

            # per-pair-block projection enqueue schedule (ready just in time;
            # pair-block 0 instead emits immediate whole-unit bursts)
            pb_enqueue = {
                1: [("qt", 0, 2)] + [("kt", 1, j) for j in range(4)],
                2: [("qt", 0, 3)] + [("kt", 1, j) for j in range(4, 8)],
                3: [("qt", 1, 0), ("qt", 1, 1)],
                4: [("qt", 1, 2), ("qt", 1, 3)],
            }

            pending_final = None

            for pb_idx, (h_lo, h_hi, j_lo, j_hi) in enumerate(PBS):
                for item in pb_enqueue.get(pb_idx, []):
                    enqueue_proj(*item)
                cx_lo = CP.tile([65, 512], f32, tag="cxlo", name="cxlo")
                cx_hi = CP.tile([65, 512], f32, tag="cxhi", name="cxhi")
                pts = []

                def emit_ctx(g, pts=pts, cx_lo=cx_lo, cx_hi=cx_hi,
                             h_lo=h_lo, h_hi=h_hi):
                    pt = pts[g]
                    nc.tensor.matmul(
                        cx_lo[:],
                        vv[:, g * VC + h_lo * 65 : g * VC + h_lo * 65 + 65],
                        pt[:, 0:512],
                        start=(g == 0),
                        stop=(g == NT - 1),
                    )
                    nc.tensor.matmul(
                        cx_hi[:],
                        vv[:, g * VC + h_hi * 65 : g * VC + h_hi * 65 + 65],
                        pt[:, 512:1024],
                        start=(g == 0),
                        stop=(g == NT - 1),
                    )

                for t in range(NT):
                    # scores for key chunk t, both paired head-blocks, as
                    # plain full-128 contractions (zero halves contribute
                    # nothing): lo block -> cols 0:512, hi block -> 512:1024
                    sc = SCP.tile([128, 1024], f32, tag="sc", name="sc")
                    nc.tensor.matmul(
                        sc[:, 0:512],
                        kts[h_lo][:, t * 128 : (t + 1) * 128],
                        qts[h_lo][:, j_lo * 512 : (j_lo + 1) * 512],
                        start=True,
                        stop=True,
                    )
                    nc.tensor.matmul(
                        sc[:, 512:1024],
                        kts[h_hi][:, t * 128 : (t + 1) * 128],
                        qts[h_hi][:, j_hi * 512 : (j_hi + 1) * 512],
                        start=True,
                        stop=True,
                    )
                    pt = WK.tile([128, 1024], bf16, tag="pt", name="pt")
                    nc.scalar.activation(pt[:], sc[:], EXP, scale=0.125)
                    pts.append(pt)
                    if t == 0 and pending_final is not None:
                        pending_final()
                        pending_final = None
                    emit_out_stage()
                    if t == 0:
                        emit_out_stage()  # free both cx banks right away
                    # interleave projections/V into the activation-bound
                    # steady state (after the exp emission so scores are
                    # never delayed behind projection work)
                    if pb_idx == 0:
                        v_unit(t)
                        if t % 2 == 0 and t // 2 + 1 <= 7:
                            qk_unit("kt", 0, t // 2 + 1)
                        if t == 24:
                            qk_unit("qt", 0, 1)
                    else:
                        proj_step()
                    # ctx runs one chunk behind exp so the PE overlaps the
                    # activation latency with the previous chunk's ctx
                    if t > 0:
                        emit_ctx(t - 1)
                # final chunk's ctx is deferred into the next block so the
                # transition never stalls on the last exp
                pending_final = (lambda f=emit_ctx: f(NT - 1))
                out_stage_q.append((j_lo, h_lo, cx_lo, {"step": 0}))
                out_stage_q.append((j_hi, h_hi, cx_hi, {"step": 0}))
            if pending_final is not None:
                pending_final()
                pending_final = None
            flush_out_stages()

    nc.compile()
    return nc


def _get_nc():
    if "nc" not in _CACHE:
        _CACHE["nc"] = _build()
    return _CACHE["nc"]


def _in_maps(hs, mask, Wq, bq, Wk, bk, Wv, bv):
    ident = np.eye(128, dtype=np.float32)

    def qk_chunks(W, hg):  # [768, :] f32 -> [128, 6*256] bf16: [h0|h1|h2|0]
        out = np.zeros((128, NHC * WCC), bf16np)
        for c in range(NHC):
            out[:, c * WCC : c * WCC + CC] = W[
                c * 128 : (c + 1) * 128, hg * CC : (hg + 1) * CC
            ].astype(bf16np)
        return out

    def v_chunks(W):  # augmented V weights -> [128, 6*195] bf16
        out = np.empty((128, NHC * VC), bf16np)
        for c in range(NHC):
            out[:, c * VC : (c + 1) * VC] = W[c * 128 : (c + 1) * 128, :].astype(
                bf16np
            )
        return out

    # per query-half: key order permuted so own queries are keys 0:2048
    m32 = mask.reshape(NT, 128)
    hsT_sh = []
    maskt_sh = []
    for sh in range(QS):
        perm = np.roll(np.arange(S), -sh * SQ)
        hsT_sh.append(np.ascontiguousarray(hs[perm, :].astype(bf16np).T))
        maskt_sh.append(
            np.ascontiguousarray(np.roll(m32, -sh * (NT // QS), axis=0).T)
        )

    maps = []
    for core in range(N_CORES):
        hg, sh = core // QS, core % QS
        wv_aug = np.zeros((HID, VC), np.float32)
        bv_aug = np.zeros((1, VC), np.float32)
        for h in range(HPC):
            wv_aug[:, h * 65 : h * 65 + 64] = Wv[
                :, hg * CC + h * 64 : hg * CC + (h + 1) * 64
            ]
            bv_aug[0, h * 65 : h * 65 + 64] = bv[
                hg * CC + h * 64 : hg * CC + (h + 1) * 64
            ]
            bv_aug[0, h * 65 + 64] = 1.0
        # per-head bias columns: col 0 = h0 (lower half), col 1 = h1 (upper
        # half), col 2 = h2 (lower half; upper stays zero like its weights)
        bqt = np.zeros((128, HPC), np.float32)
        bkt = np.zeros((128, HPC), np.float32)
        for h, lo in ((0, 0), (1, 64), (2, 0)):
            bqt[lo : lo + 64, h] = bq[hg * CC + h * 64 : hg * CC + (h + 1) * 64]
            bkt[lo : lo + 64, h] = bk[hg * CC + h * 64 : hg * CC + (h + 1) * 64]
        maps.append(
            {
                "hsT": hsT_sh[sh],
                "wqb": qk_chunks(Wq, hg),
                "wkb": qk_chunks(Wk, hg),
                "wvb": v_chunks(wv_aug),
                "bqt": bqt,
                "bkt": bkt,
                "bvb": bv_aug.astype(bf16np),
                "maskt": maskt_sh[sh],
                "ident": ident,
            }
        )
    return maps


def kernel(hidden_states, attention_mask, Wq, bq, Wk, bk, Wv, bv, **run_kwargs):
    hs = np.ascontiguousarray(np.asarray(hidden_states, np.float32).reshape(S, HID))
    mask = np.ascontiguousarray(np.asarray(attention_mask, np.float32).reshape(S))
    Wq = np.asarray(Wq, np.float32)
    Wk = np.asarray(Wk, np.float32)
    Wv = np.asarray(Wv, np.float32)
    bq = np.asarray(bq, np.float32)
    bk = np.asarray(bk, np.float32)
    bv = np.asarray(bv, np.float32)

    nc = _get_nc()
    maps = _in_maps(hs, mask, Wq, bq, Wk, bk, Wv, bv)
    res = bass_utils.run_bass_kernel_spmd(
        nc, maps, core_ids=list(range(N_CORES)), **run_kwargs
    )
    out = np.zeros((S, NH * HD), np.float32)
    for core in range(N_CORES):
        hg, sh = core // QS, core % QS
        out[sh * SQ : (sh + 1) * SQ, hg * CC : (hg + 1) * CC] = res.results[core][
            "out"
        ]
    if "trace" in run_kwargs:
        _CACHE["last_result"] = res
    return out.reshape(B, S, NH * HD)
